# revision 8
# baseline (speedup 1.0000x reference)
"""Trainium2 Bass kernel for the NumReps masked-mean problem.

Math: each mask row is a contiguous run of ones (1..8 long). expand_window
widens it by int(0.2*len) (== 1 iff len >= 5) on each side, clamped to
[0, S-1]; the output row is the mean of reps rows over the widened window
(window length n is in {1,2,3,4} u {6,7,8,9,10}; n=6 only when the window
clamps at 0).

Strategy (per core, data-parallel over batch: 16 batches / 8 cores = 2):
  - run length via scalar-engine accumulate, position-sum via fused
    scalar_tensor_tensor passes over the mask, split into 2 S-halves so
    they overlap the mask DMA
  - ONE combined index chain for both batches on [128,2] tiles (first
    recovered exactly via the 2^23+2^22 magic rint; f32->i32 conversion
    rounds to nearest so indices must be exact); all 6 gather indices
    produced in one [128,6] tile (out-of-range +4096 trick for
    conditionally-skipped chunks)
  - split indirect-DMA gather per batch: chunks {0..3} always, {4..9} for
    n >= 6 (heads dispatched first); skipped landing areas are pre-zeroed
    at kernel start, spread across DVE/ACT/gpsimd which are idle then
  - weighted windowed sum on the TensorEngine: 10 accumulating diagonal
    matmuls per batch in float32r (full rate at N=512), diag_j =
    diag((j<n)/n); gathered chunks staged through f32r tiles (rounding-op
    source required by the BIR verifier), casts split DVE/ACT (gpsimd is
    ~8x slower at casts - never cast there)
  - PSUM -> SBUF copies (DVE + ACT), store
"""

import numpy as np

B, M, S, D = 16, 128, 2048, 1024
NCORES = 8
BPC = B // NCORES  # batches per core
WMAX = 10  # max expanded window length

# gather plan: (chunk_start, n_chunks, n-threshold or None=always)
GATHER_PLAN = [(0, 4, None), (4, 6, 4.5)]
ZCHUNK = 4  # first chunk whose landing area needs pre-zeroing
RINT_MAGIC = 12582912.0  # 2^23 + 2^22

_cache = {}


def _build_nc():
    import concourse.bacc as bacc
    import concourse.bass as bass
    import concourse.mybir as mybir
    from concourse import tile

    f32 = mybir.dt.float32
    f32r = mybir.dt.float32r
    i16 = mybir.dt.int16
    i32 = mybir.dt.int32
    Alu = mybir.AluOpType
    Act = mybir.ActivationFunctionType

    nc = bacc.Bacc("TRN2", target_bir_lowering=False, debug=False)

    mask = nc.dram_tensor("mask", [BPC, M, S], f32, kind="ExternalInput")
    reps = [
        nc.dram_tensor(f"reps{b}", [S, D], f32, kind="ExternalInput")
        for b in range(BPC)
    ]
    out = nc.dram_tensor("out", [BPC, M, D], f32, kind="ExternalOutput")

    K = len(GATHER_PLAN)
    BK = BPC * K
    H = S // 2

    # inline consts (their DMA runs in the dead window before program start)
    iota_np = np.broadcast_to(np.arange(S, dtype=np.int16), (M, S))
    iota_const = nc.inline_tensor(np.ascontiguousarray(iota_np),
                                  name="iota_const")
    thr_np = np.array([(-1.0 if th is None else th)
                       for (_, _, th) in GATHER_PLAN] * BPC, dtype=np.float32)
    offc_np = np.array([cs + 4096.0 for (cs, _, _) in GATHER_PLAN] * BPC,
                       dtype=np.float32)
    tc_np = np.broadcast_to(
        np.concatenate([thr_np, offc_np])[None, :], (M, 2 * BK))
    tc_const = nc.inline_tensor(np.ascontiguousarray(tc_np), name="tc_const")

    with tile.TileContext(nc) as tc:
        with (
            tc.tile_pool(name="const", bufs=1) as cpool,
            tc.tile_pool(name="big", bufs=2) as big,
            tc.tile_pool(name="small", bufs=2) as small,
            tc.tile_pool(name="psum", bufs=2, space="PSUM") as psum,
        ):
            # DMA issue order puts the first-needed bytes first: mask
            # b0-h0, iota (needed by the first possum pass), then the rest
            mts = []
            for b in range(BPC):
                mt = big.tile([M, S], f32, tag=f"mask{b}", name=f"mask{b}")
                mts.append(mt)
            nc.sync.dma_start(mts[0][:, :H], mask[0][:, :H])
            iota_f = cpool.tile([M, S], i16)
            nc.sync.dma_start(iota_f[:], iota_const[:])
            nc.sync.dma_start(mts[1][:, :H], mask[1][:, :H])
            nc.sync.dma_start(mts[0][:, H:], mask[0][:, H:])
            nc.sync.dma_start(mts[1][:, H:], mask[1][:, H:])
            toc = cpool.tile([M, 2 * BK], f32)
            nc.sync.dma_start(toc[:], tc_const[:])
            thr6 = toc[:, :BK]
            offc6 = toc[:, BK:]

            ident = cpool.tile([M, M], f32)
            nc.gpsimd.memset(ident[:], 1.0)
            nc.gpsimd.affine_select(
                out=ident[:], in_=ident[:], compare_op=Alu.is_equal,
                fill=0.0, base=0, pattern=[[-1, M]], channel_multiplier=1,
            )

            # explicit gather tiles (one per batch); conditional tail areas
            # pre-zeroed once, spread across engines idle at t=0
            gts = [
                cpool.tile([M, WMAX * D], f32, tag=f"gt{b}", name=f"gt{b}")
                for b in range(BPC)
            ]
            nc.vector.memset(gts[0][:, ZCHUNK * D:8 * D], 0.0)
            nc.scalar.memzero(gts[0][:, 8 * D:])
            nc.gpsimd.memset(gts[1][:, ZCHUNK * D:8 * D], 0.0)
            nc.scalar.memzero(gts[1][:, 8 * D:])

            # len on ACT (accum of mask), possum partial on DVE; elementwise
            # outs land in stride-0 one-column sinks. lh/ah columns are
            # (b0h0, b1h0, b0h1, b1h1)
            lsink = small.tile([M, 1], f32, tag="lsink")
            lh = small.tile([M, 4], f32, tag="lh")
            lsink_ap = bass.AP(
                lsink[:].tensor, lsink[:].offset, [lsink[:].ap[0], [0, H]]
            )
            ssink = small.tile([M, 1], f32, tag="ssink")
            ah = small.tile([M, 4], f32, tag="ah")
            ssink_ap = bass.AP(
                ssink[:].tensor, ssink[:].offset, [ssink[:].ap[0], [0, H]]
            )
            for h in range(2):
                for b in range(BPC):
                    nc.scalar.activation(
                        out=lsink_ap, in_=mts[b][:, h * H:(h + 1) * H],
                        func=Act.Identity,
                        accum_out=lh[:, 2 * h + b:2 * h + b + 1],
                    )
                    nc.vector.scalar_tensor_tensor(
                        out=ssink_ap, in0=iota_f[:, h * H:(h + 1) * H],
                        scalar=-4096.0, in1=mts[b][:, h * H:(h + 1) * H],
                        op0=Alu.add, op1=Alu.mult,
                        accum_out=ah[:, 2 * h + b:2 * h + b + 1],
                    )

            # combined chain for both batches on [M, 2] tiles
            lenf = small.tile([M, 2], f32, tag="lenf")
            nc.vector.tensor_tensor(
                out=lenf[:], in0=lh[:, :2], in1=lh[:, 2:4], op=Alu.add)
            a1 = small.tile([M, 2], f32, tag="a1")
            nc.vector.tensor_tensor(
                out=a1[:], in0=ah[:, :2], in1=ah[:, 2:4], op=Alu.add)
            # possum = a1 + 4096*len (exact)
            psm = small.tile([M, 2], f32, tag="psm")
            nc.vector.scalar_tensor_tensor(
                out=psm[:], in0=lenf[:], scalar=4096.0, in1=a1[:],
                op0=Alu.mult, op1=Alu.add)
            # first = rint(possum/len - (len-1)/2)
            rl = small.tile([M, 2], f32, tag="rl")
            nc.vector.reciprocal(rl[:], lenf[:])
            hl = small.tile([M, 2], f32, tag="hl")
            nc.vector.tensor_scalar(
                out=hl[:], in0=lenf[:], scalar1=-1.0, scalar2=0.5,
                op0=Alu.add, op1=Alu.mult)
            first = small.tile([M, 2], f32, tag="first")
            nc.vector.tensor_tensor(
                out=first[:], in0=psm[:], in1=rl[:], op=Alu.mult)
            nc.vector.tensor_tensor(
                out=first[:], in0=first[:], in1=hl[:], op=Alu.subtract)
            nc.vector.tensor_scalar(
                out=first[:], in0=first[:], scalar1=RINT_MAGIC,
                scalar2=-RINT_MAGIC, op0=Alu.add, op1=Alu.add)
            # e = 1 iff len >= 5; ns = max(first-e, 0);
            # ne = min(first+len+e-1, S-1); n = ne-ns+1
            e = small.tile([M, 2], f32, tag="e")
            nc.vector.tensor_scalar(
                out=e[:], in0=lenf[:], scalar1=4.5, scalar2=None,
                op0=Alu.is_ge)
            ns = small.tile([M, 2], f32, tag="ns")
            nc.vector.tensor_tensor(
                out=ns[:], in0=first[:], in1=e[:], op=Alu.subtract)
            nc.vector.tensor_scalar(
                out=ns[:], in0=ns[:], scalar1=0.0, scalar2=None, op0=Alu.max)
            t_ = small.tile([M, 2], f32, tag="t_")
            nc.vector.tensor_tensor(
                out=t_[:], in0=lenf[:], in1=e[:], op=Alu.add)
            ne = small.tile([M, 2], f32, tag="ne")
            nc.vector.tensor_tensor(
                out=ne[:], in0=first[:], in1=t_[:], op=Alu.add)
            nc.vector.tensor_scalar(
                out=ne[:], in0=ne[:], scalar1=-1.0, scalar2=float(S - 1),
                op0=Alu.add, op1=Alu.min)
            n = small.tile([M, 2], f32, tag="n")
            nc.vector.tensor_tensor(
                out=n[:], in0=ne[:], in1=ns[:], op=Alu.subtract)
            nc.vector.tensor_scalar_add(n[:], n[:], 1.0)
            inv = small.tile([M, 2], f32, tag="inv")
            nc.vector.reciprocal(inv[:], n[:])

            # all 6 gather indices at once (b-major columns):
            # idx = cv*(-4096) + (ns + start + 4096)
            cv = small.tile([M, BK], f32, tag="cv")
            nc.vector.tensor_tensor(
                out=cv[:].rearrange("p (b k) -> p b k", b=BPC),
                in0=n[:].unsqueeze(-1).to_broadcast([M, BPC, K]),
                in1=thr6.rearrange("p (b k) -> p b k", b=BPC),
                op=Alu.is_ge)
            qv = small.tile([M, BK], f32, tag="qv")
            nc.vector.tensor_tensor(
                out=qv[:].rearrange("p (b k) -> p b k", b=BPC),
                in0=ns[:].unsqueeze(-1).to_broadcast([M, BPC, K]),
                in1=offc6.rearrange("p (b k) -> p b k", b=BPC),
                op=Alu.add)
            idxf = small.tile([M, BK], f32, tag="idxf")
            nc.vector.scalar_tensor_tensor(
                out=idxf[:], in0=cv[:], scalar=-4096.0, in1=qv[:],
                op0=Alu.mult, op1=Alu.add)
            idxi = small.tile([M, BK], i32, tag="idxi")
            nc.vector.tensor_copy(idxi[:], idxf[:])

            # gathers (skipped descriptors land on the pre-zeroed area);
            # heads dispatched before tails so the matmul pipe starts early
            for k, (cs, nch, th) in enumerate(GATHER_PLAN):
                for b in range(BPC):
                    kw = {}
                    if th is not None:
                        kw = dict(bounds_check=S - 1, oob_is_err=False)
                    nc.gpsimd.indirect_dma_start(
                        out=gts[b][:, cs * D:(cs + nch) * D],
                        out_offset=None,
                        in_=reps[b][:],
                        in_offset=bass.IndirectOffsetOnAxis(
                            ap=idxi[:, b * K + k:b * K + k + 1], axis=0),
                        **kw,
                    )

            # weights + diag set per batch
            diags = []
            for b in range(BPC):
                w = small.tile([M, WMAX], f32, tag="w")
                nc.vector.tensor_scalar(
                    out=w[:], in0=iota_f[:, :WMAX], scalar1=n[:, b:b + 1],
                    scalar2=inv[:, b:b + 1], op0=Alu.is_lt, op1=Alu.mult)
                diag = big.tile([M, WMAX * M], f32r, tag="diag")
                nc.vector.tensor_tensor(
                    out=diag[:].rearrange("p (j q) -> p j q", j=WMAX),
                    in0=ident[:].unsqueeze(1).to_broadcast([M, WMAX, M]),
                    in1=w[:].unsqueeze(-1).to_broadcast([M, WMAX, M]),
                    op=Alu.mult,
                )
                diags.append(diag)

            # staged casts + matmuls, interleaved across batches in chunk
            # order so both PE pipelines start as soon as head data lands.
            # fp32r rhs must be produced by a rounding op (the verifier keys
            # on the memory location, so the DMA-written gather tile can't
            # feed the PE directly). Cast engine: DVE for head chunks and
            # the last pair, DVE/ACT alternating for the tail pairs.
            cast_eng = {
                (0, 0): "v", (0, 1): "v", (0, 2): "a", (0, 3): "v", (0, 4): "v",
                (1, 0): "v", (1, 1): "v", (1, 2): "v", (1, 3): "a", (1, 4): "v",
            }
            pss = []
            osums = []
            for b in range(BPC):
                osums.append(big.tile([M, D], f32, tag="osum",
                                      name=f"osum{b}"))
                pss.append((psum.tile([M, 512], f32, tag="ps0",
                                      name=f"ps0_{b}"),
                            psum.tile([M, 512], f32, tag="ps1",
                                      name=f"ps1_{b}")))
            for h in range(WMAX // 2):
                for b in range(BPC):
                    gtr = big.tile([M, 2 * D], f32r, tag="gtr", bufs=4,
                                   name=f"gtr_{b}_{h}")
                    src_ap = gts[b][:, 2 * h * D:(2 * h + 2) * D]
                    if cast_eng[(b, h)] == "v":
                        nc.vector.tensor_copy(gtr[:], src_ap)
                    else:
                        nc.scalar.copy(gtr[:], src_ap)
                    ps0, ps1 = pss[b]
                    for j in (2 * h, 2 * h + 1):
                        dj = diags[b][:, j * M:(j + 1) * M]
                        seg = gtr[:, (j % 2) * D:(j % 2 + 1) * D]
                        nc.tensor.matmul(
                            ps0[:], lhsT=dj, rhs=seg[:, :512],
                            start=(j == 0), stop=(j == WMAX - 1),
                        )
                        nc.tensor.matmul(
                            ps1[:], lhsT=dj, rhs=seg[:, 512:],
                            start=(j == 0), stop=(j == WMAX - 1),
                        )
            for b in range(BPC):
                ps0, ps1 = pss[b]
                nc.vector.tensor_copy(osums[b][:, :512], ps0[:])
                nc.vector.tensor_copy(osums[b][:, 512:], ps1[:])
                nc.sync.dma_start(out[b], osums[b][:])

    nc.finalize()
    return nc


def _get_nc():
    if "nc" not in _cache:
        _cache["nc"] = _build_nc()
    return _cache["nc"]


def _shard_inputs(number_mask, reps):
    in_maps = []
    for c in range(NCORES):
        m = {"mask": np.ascontiguousarray(number_mask[c * BPC:(c + 1) * BPC])}
        for b in range(BPC):
            m[f"reps{b}"] = np.ascontiguousarray(reps[c * BPC + b])
        in_maps.append(m)
    return in_maps


def _install_ntff_hook():
    """The image's antenv lacks axon_hooks; synthesize it so trace=True
    (NTFF profiling) works through run_bass_kernel_spmd."""
    import sys
    import types

    try:
        from antenv.axon_hooks import get_axon_ntff_profile_hook  # noqa: F401
        return
    except ImportError:
        pass
    from trn_agent_boot.trn_boot import _ntff_profile_via_ctypes

    mod = types.ModuleType("antenv.axon_hooks")
    _hook = [_ntff_profile_via_ctypes("/opt/axon/libaxon_pjrt.so")]
    mod.get_axon_ntff_profile_hook = lambda: _hook[0]
    mod.set_axon_ntff_profile_hook = lambda h: _hook.__setitem__(0, h)
    sys.modules["antenv.axon_hooks"] = mod
    import antenv

    antenv.axon_hooks = mod


def _run(number_mask, reps, trace=False):
    from concourse.bass_utils import run_bass_kernel_spmd

    if trace:
        _install_ntff_hook()
    nc = _get_nc()
    in_maps = _shard_inputs(number_mask, reps)
    res = run_bass_kernel_spmd(
        nc, in_maps, core_ids=list(range(NCORES)), trace=trace
    )
    outs = np.stack([r["out"] for r in res.results], axis=0)
    return outs.reshape(B, M, D), res


def kernel(**inputs):
    out, _ = _run(inputs["number_mask"], inputs["reps"], trace=False)
    return out
